# revision 53
# baseline (speedup 1.0000x reference)
"""Trainium2 Bass kernel: multi-head attention with quantum (cumprod-of-cos) transform.

Full-input contract: kernel(**inputs) takes the unsharded inputs and returns the
full [B, S, E] output. Internally shards over 8 NeuronCores: data-parallel over
batch (B=2) x tensor-parallel over head-groups (4 heads per core).

Per-core pipeline (b = batch, g = head-group of 4 heads, EG = 256 e-dims):
  x8/w8: x and Wq/Wk/Wv arrive pre-quantized to fp8e4 from the host.
  theta [s, f] per t-tile: fp8 DoubleRow matmuls pairing the 8 contraction
      tiles (2 k-tiles per instruction, 2 fp8 rows/cycle on HW). The q
      projection runs as per-contraction-pair passes over all 16 t-tiles so
      the PE starts as soon as the first two x tiles land.
  c = cos(theta) via ACT Sin(theta + pi/2); all Sin ops precede all Exp ops
      in ACT program order -> exactly 2 activation-table loads.
  z = cumprod(c) along d via DVE tensor_tensor_scan (mult/bypass, fp32 state),
      one 64-wide scan per head: q,k -> bf16; v -> fp8e4 (+ fp8 ones columns
      interleaved for the free softmax denominator).
  qzT/kzT: PE transpose (bf16 identity matmul) [s,e] -> [e,s] between the k
      and v projections; PSUM->SBUF copies split across ACT and DVE.
  scores^T [t, s] per head/t-tile = kzT-tile (stationary) x qzT (moving), bf16.
  ex8 = exp(scores/8 + ln2/2) in fp8e4: 10 tiles/iter on ACT (spline exp, fp8
      out) and 6 on DVE (one tensor_scalar: bits = round(s/ln2 + 60) written
      as uint8 == the e4m3 bit pattern; |err| ~ fp8 quantization). Odd tiles
      on DVE so consecutive tiles drain on different engines.
  out-matmul: fp8 DoubleRow pairing two t-tiles per instruction; stationary
      [ones | vz8] -> acc rows 0:64 = softmax denominator (reciprocal_approx_
      fast needs partition-0-based input), rows 64:128 = unnormalized out^T.
      oz = acc[64:128] * recip(acc[0:64]) on DVE.
  yT_partial [E, S] = WcT_slice (stationary, f32r) x oz -- host sums 4
      partials. Software-pipelined attention: iteration i emits scores+exp,
      then iteration i-1's out-matmuls; the final projection of s-half 0 is
      split across two pipeline slots to smooth the PSUM rotation.
"""

import os
import sys

import numpy as np

if "/opt/trn_rl_repo" not in sys.path:
    sys.path.insert(0, "/opt/trn_rl_repo")

import concourse.bass as bass  # noqa: F401
import concourse.tile as tile
from concourse import bacc
from concourse import mybir
from concourse.bass_utils import run_bass_kernel_spmd

AF = mybir.ActivationFunctionType
ALU = mybir.AluOpType
F32 = mybir.dt.float32
F32R = mybir.dt.float32r
BF16 = mybir.dt.bfloat16
F8 = mybir.dt.float8e4
U8 = mybir.dt.uint8

B, S, E, H, D = 2, 2048, 1024, 16, 64
NCORES = 8
HG = 4          # heads per core
EG = HG * D     # 256
P = 128
NT = S // P     # 16 t-tiles
KC = E // P     # 8 contraction tiles for the projections
HALF_PI = float(np.pi / 2)
INV_SQRT_D = 0.125  # 1/sqrt(64)
# ex = exp(s/8 + ln2/2): raw scores measured in [-6.8, 11.3] so max ex = 5.9
# and the DVE bit pattern round(s*1/ln2 + 60) stays in [50, 77] -- far from
# the uint8 wrap at 0 and the e4m3 NaN zone above 126. The uniform 2^(1/2)
# scale cancels against the matching denominator.
EXP_SHIFT = float(np.log(2.0) / 2.0)
EXP_A = float(1.0 / np.log(2.0))
EXP_B = 60.0
# exp split: odd t-tiles on DVE so each (even, odd) pair overlaps ACT||DVE
_DVE_T = frozenset((1, 3, 5, 7, 9, 11))
_DEBUG = bool(int(os.environ.get("QK_DEBUG", "0")))


def _projections_q(tc, x8, w8q, c_pool, z_pool, zqk, psQ, hp):
    """q projection as contraction passes: pass j touches all 16 t-tiles, so
    the first matmuls need only x8[:, 0:2] instead of the full x8 -- the PE
    starts ~10us earlier while the remaining x tiles stream in."""
    nc = tc.nc
    for half in range(2):
        thh = psQ.tile([P, KC, EG], F32, tag="thh", bufs=2, name=f"thh{half}")
        for j in range(KC // 2):
            for ts in range(KC):
                t = half * KC + ts
                nc.tensor.matmul(
                    thh[:, ts, :],
                    lhsT=x8[:, 2 * j:2 * j + 2, t * P:(t + 1) * P],
                    rhs=w8q[:, 2 * j:2 * j + 2, :],
                    start=(j == 0), stop=(j == KC // 2 - 1),
                    perf_mode=mybir.MatmulPerfMode.DoubleRow,
                )
        for u in range(KC // 2):
            c_t = c_pool.tile([P, 2 * EG], F32, tag="c", name=f"cq{half}{u}")
            nc.scalar.activation(c_t[:], thh[:, 2 * u:2 * u + 2, :], AF.Sin,
                                 bias=hp[:])
            for sl in range(2):
                t = half * KC + 2 * u + sl
                zt = z_pool.tile([P, EG], BF16, tag=f"zq{t}", name=f"zq{t}")
                zqk[("q", t)] = zt
                for h in range(HG):
                    src = c_t[:, sl * EG + h * D:sl * EG + (h + 1) * D]
                    nc.vector.tensor_tensor_scan(
                        out=zt[:, h * D:(h + 1) * D], data0=src, data1=src,
                        initial=1.0, op0=ALU.mult, op1=ALU.bypass,
                    )


def _projections(tc, x8, w8s, c_pool, z_pool, vz8, zqk, ps, hp, names):
    """theta -> cos -> cumprod. Emits ACT Sin ops (all Sins precede all
    Exps across calls)."""
    nc = tc.nc
    for name in names:
        w8 = w8s[name]
        for tp in range(NT // 2):
            th = ps.tile([P, 2 * EG], F32, tag="th", bufs=4, name=f"th{name}{tp}")
            for sl in range(2):
                t = 2 * tp + sl
                for j in range(KC // 2):
                    nc.tensor.matmul(
                        th[:, sl * EG:(sl + 1) * EG],
                        lhsT=x8[:, 2 * j:2 * j + 2, t * P:(t + 1) * P],
                        rhs=w8[:, 2 * j:2 * j + 2, :],
                        start=(j == 0), stop=(j == KC // 2 - 1),
                        perf_mode=mybir.MatmulPerfMode.DoubleRow,
                    )
            c_t = c_pool.tile([P, 2 * EG], F32, tag="c", name=f"c{name}{tp}")
            nc.scalar.activation(c_t[:], th[:], AF.Sin, bias=hp[:])
            for sl in range(2):
                t = 2 * tp + sl
                for h in range(HG):
                    off = sl * EG + h * D
                    src = c_t[:, off:off + D]
                    if name == "v":
                        out = vz8[tp][:, sl, 2 * h + 1, :]
                    else:
                        zt = zqk[(name, t)] = (
                            zqk.get((name, t))
                            or z_pool.tile([P, EG], BF16, tag=f"z{name}{t}",
                                           name=f"z{name}{t}")
                        )
                        out = zt[:, h * D:(h + 1) * D]
                    nc.vector.tensor_tensor_scan(
                        out=out, data0=src, data1=src,
                        initial=1.0, op0=ALU.mult, op1=ALU.bypass,
                    )


def _transposes(tc, zqk, ident, zT, ps):
    """[s, e] bf16 tiles -> [e, s] via PE transpose; PSUM->SBUF copies split
    ACT/DVE. Runs between k- and v-projections so v's matmuls keep the PE fed
    while the last scans drain."""
    nc = tc.nc
    for name in ("q", "k"):
        for t in range(NT):
            for m in range(2):
                pt = ps.tile([P, P], BF16, tag="tp", bufs=4,
                             name=f"pt{name}{m}{t}")
                nc.tensor.matmul(
                    pt[:], lhsT=zqk[(name, t)][:, m * P:(m + 1) * P],
                    rhs=ident[:], is_transpose=True, start=True, stop=True,
                )
                eng = nc.scalar if t % 2 == 0 else nc.vector
                if eng is nc.scalar:
                    nc.scalar.copy(out=zT[(name, m)][:, t * P:(t + 1) * P],
                                   in_=pt[:])
                else:
                    nc.vector.tensor_copy(
                        out=zT[(name, m)][:, t * P:(t + 1) * P], in_=pt[:])


def _attention_and_final(tc, zT, vz8, wc_t, oz_pool, yT, psB, esh, dbg):
    nc = tc.nc
    with (
        tc.tile_pool(name="exps", bufs=3) as exq,
        tc.tile_pool(name="norm", bufs=2) as nrm,
        tc.tile_pool(name="y", bufs=3) as yp,
    ):
        ozs = {}

        def emit_outdr(sb, h, exs):
            # out-matmuls + normalize for a (sb, h) whose exps are in flight
            ssl0 = sb * 1024
            m = h // 2
            dbase = (h % 2) * D
            acc = psB.tile([P, 1024], F32, tag="acc", bufs=1, name=f"acc{h}_{sb}")
            for tp in range(NT // 2):
                for ch in range(2):
                    nc.tensor.matmul(
                        acc[:, ch * 512:(ch + 1) * 512],
                        lhsT=vz8[tp][:, :, 2 * h:2 * h + 2, :].rearrange(
                            "p a b d -> p a (b d)"),
                        rhs=exs[tp][:, :, ch * 512:(ch + 1) * 512],
                        start=(tp == 0), stop=(tp == NT // 2 - 1),
                        perf_mode=mybir.MatmulPerfMode.DoubleRow,
                    )
            if _DEBUG and sb == 0 and h == 0:
                accs = nrm.tile([P, 1024], F32, tag="accs", name=f"accs{h}{sb}")
                nc.vector.tensor_copy(out=accs[:], in_=acc[:])
                nc.sync.dma_start(out=dbg["dbg_acc00"][:], in_=accs[:])
            rec = nrm.tile([D, 1024], F32, tag="rec", name=f"rec{h}{sb}")
            nc.vector.reciprocal_approx_fast(rec[:], acc[0:D, :])
            nc.vector.tensor_tensor(
                out=ozs[m][dbase:dbase + D, ssl0:ssl0 + 1024],
                in0=acc[D:2 * D, :], in1=rec[:], op=ALU.mult,
            )

        def emit_final(sb, mos=range(E // P)):
            for mo in mos:
                py = psB.tile([P, 1024], F32, tag="s", bufs=3, name=f"py{mo}{sb}")
                for ch in range(2):
                    ssl = slice(sb * 1024 + ch * 512, sb * 1024 + (ch + 1) * 512)
                    for kk in range(2):
                        nc.tensor.matmul(
                            py[:, ch * 512:(ch + 1) * 512],
                            lhsT=wc_t[:, kk, mo * P:(mo + 1) * P],
                            rhs=ozs[kk][:, ssl],
                            start=(kk == 0), stop=(kk == 1),
                        )
                yt = yp.tile([P, 1024], F32, tag="y", name=f"yt{mo}{sb}")
                if mo % 2 == 0:
                    nc.scalar.copy(out=yt[:], in_=py[:])
                else:
                    nc.vector.tensor_copy(out=yt[:], in_=py[:])
                nc.sync.dma_start(
                    out=yT[mo * P:(mo + 1) * P, sb * 1024:(sb + 1) * 1024],
                    in_=yt[:],
                )

        prev = None
        iters = [(sb, h) for sb in range(2) for h in range(HG)]
        for i, (sb, h) in enumerate(iters):
            ssl0 = sb * 1024
            m = h // 2
            dbase = (h % 2) * D
            qzT = zT[("q", m)]
            kzT = zT[("k", m)]
            if sb == 0 and (h % 2) == 0:
                ozm = oz_pool.tile([P, S], F32R, tag=f"oz{m}", name=f"oz{m}")
                ozs[m] = ozm
                _OZ_HANDLES[m] = ozm
            exs = []
            for tp in range(NT // 2):
                ex = exq.tile([P, 2, 1024], F8, tag="ex", bufs=18,
                              name=f"ex{h}_{tp}{sb}")
                exs.append(ex)
                if _DEBUG and sb == 0 and h == 0 and tp in (0, 3):
                    dbg[("ex", tp)] = ex
                for sl in range(2):
                    t = 2 * tp + sl
                    sc = psB.tile([P, 1024], F32, tag="s", bufs=3,
                                  name=f"sc{h}_{t}{sb}")
                    for ch in range(2):
                        nc.tensor.matmul(
                            sc[:, ch * 512:(ch + 1) * 512],
                            lhsT=kzT[dbase:dbase + D, t * P:(t + 1) * P],
                            rhs=qzT[dbase:dbase + D,
                                    ssl0 + ch * 512:ssl0 + (ch + 1) * 512],
                            start=True, stop=True,
                        )
                    if t in _DVE_T:
                        # DVE path: e4m3 bit pattern via one affine+round
                        nc.vector.tensor_scalar(
                            out=ex[:, sl, :].bitcast(U8), in0=sc[:],
                            scalar1=EXP_A, scalar2=EXP_B,
                            op0=ALU.mult, op1=ALU.add,
                        )
                    else:
                        nc.scalar.activation(
                            ex[:, sl, :], sc[:], AF.Exp,
                            scale=INV_SQRT_D, bias=esh[:],
                        )
                if _DEBUG and sb == 0 and h == 0 and tp in (0, 3):
                    nc.sync.dma_start(out=dbg[f"dbg_ex{tp}"][:],
                                      in_=dbg[("ex", tp)][:])
            if prev is not None:
                emit_outdr(*prev)
            if i == 5:
                emit_final(0, range(4))    # sb 0 done after iteration 4's outdr
            elif i == 6:
                emit_final(0, range(4, 8))
            prev = (sb, h, exs)
        emit_outdr(*prev)
        emit_final(1)


_OZ_HANDLES = {}


def _build_body(tc, xT, wqT, wkT, wvT, wcT, identT, yT, dbg):
    nc = tc.nc

    with (
        tc.tile_pool(name="const", bufs=1) as const,
        tc.tile_pool(name="wc", bufs=1) as wcp,
        tc.tile_pool(name="x8p", bufs=1) as x8p,
        tc.tile_pool(name="w8p", bufs=1) as w8p,
        tc.tile_pool(name="vz", bufs=1) as vzp,
        tc.tile_pool(name="zT", bufs=1) as zTp,
        tc.tile_pool(name="outz", bufs=1) as ozp,
    ):
        ident = const.tile([P, P], BF16)
        nc.sync.dma_start(out=ident[:], in_=identT[:])
        hp = const.tile([P, 1], F32)
        nc.vector.memset(hp[:], HALF_PI)
        esh = const.tile([P, 1], F32)
        nc.vector.memset(esh[:], EXP_SHIFT)
        wc_t = wcp.tile([P, 2, E], F32R)

        x8 = x8p.tile([P, KC, S], F8)
        w8s = {}
        vz8 = [vzp.tile([P, 2, 2 * HG, D], F8, tag=f"vz{tp}", name=f"vz{tp}")
               for tp in range(NT // 2)]
        zT = {(name, m): zTp.tile([P, S], BF16, tag=f"zT{name}{m}",
                                  name=f"zT{name}{m}")
              for name in ("q", "k") for m in range(2)}

        with (
            tc.tile_pool(name="ctile", bufs=3) as cp,
            tc.tile_pool(name="ztile", bufs=1) as zp,
        ):
            # x and q/k/v weights arrive pre-quantized to fp8 from the host:
            # q weights first (the first projection matmul needs them), then
            # the x tiles in contraction order
            w8_q = w8p.tile([P, KC, EG], F8, tag="w8q", name="w8_q")
            nc.sync.dma_start(
                out=w8_q[:], in_=wqT.rearrange("(k p) e -> p k e", p=P))
            w8s["q"] = w8_q
            for k in range(KC):
                nc.sync.dma_start(out=x8[:, k, :], in_=xT[k * P:(k + 1) * P, :])
            for name, wT in (("k", wkT), ("v", wvT)):
                w8 = w8p.tile([P, KC, EG], F8, tag=f"w8{name}", name=f"w8{name}")
                w8s[name] = w8
                nc.sync.dma_start(
                    out=w8[:], in_=wT.rearrange("(k p) e -> p k e", p=P))

            nc.sync.dma_start(out=wc_t[:],
                              in_=wcT.rearrange("(k p) e -> p k e", p=P))
            # fp8 ones columns for the softmax denominator; ones FIRST within
            # each head pair so the denominator lands on out partitions 0:64
            # (reciprocal_approx_fast mishandles partition-offset inputs)
            for tp in range(NT // 2):
                nc.gpsimd.memset(vz8[tp][:, :, 0:2 * HG:2, :], 1.0)

            zqk = {}
            with tc.tile_pool(name="psQ", bufs=1, space="PSUM") as psQ:
                _projections_q(tc, x8, w8s["q"], cp, zp, zqk, psQ, hp)
            with tc.tile_pool(name="psA", bufs=1, space="PSUM") as psA:
                _projections(tc, x8, w8s, cp, zp, vz8, zqk, psA, hp, ("k",))
                _transposes(tc, zqk, ident, zT, psA)
                _projections(tc, x8, w8s, cp, zp, vz8, zqk, psA, hp, ("v",))
            if _DEBUG:
                nc.sync.dma_start(out=dbg["dbg_x8"][:], in_=x8[:])
                nc.sync.dma_start(out=dbg["dbg_w8q"][:], in_=w8s["q"][:])
                nc.sync.dma_start(out=dbg["dbg_zq0"][:], in_=zqk[("q", 0)][:])
                nc.sync.dma_start(out=dbg["dbg_zTq0"][:], in_=zT[("q", 0)][:])
                nc.sync.dma_start(out=dbg["dbg_vz0"][:], in_=vz8[0][:])

        with tc.tile_pool(name="psB", bufs=1, space="PSUM") as psB:
            _attention_and_final(tc, zT, vz8, wc_t, ozp, yT, psB, esh, dbg)
            if _DEBUG:
                nc.sync.dma_start(out=dbg["dbg_oz0"][:],
                                  in_=_OZ_HANDLES[0][:].bitcast(F32))


def build_bass():
    nc = bacc.Bacc(None, target_bir_lowering=False)
    xT = nc.dram_tensor("xT", [E, S], F8, kind="ExternalInput")
    wqT = nc.dram_tensor("wqT", [E, EG], F8, kind="ExternalInput")
    wkT = nc.dram_tensor("wkT", [E, EG], F8, kind="ExternalInput")
    wvT = nc.dram_tensor("wvT", [E, EG], F8, kind="ExternalInput")
    wcT = nc.dram_tensor("wcT", [EG, E], F32R, kind="ExternalInput")
    identT = nc.dram_tensor("identT", [P, P], BF16, kind="ExternalInput")
    yT = nc.dram_tensor("yT", [E, S], F32, kind="ExternalOutput")
    dbg = {}
    if _DEBUG:
        for nm, shp, dt in (
            ("dbg_x8", [P, KC, S], F8), ("dbg_w8q", [P, KC, EG], F8),
            ("dbg_zq0", [P, EG], BF16), ("dbg_zTq0", [P, S], BF16),
            ("dbg_vz0", [P, 2, 2 * HG, D], F8),
            ("dbg_ex0", [P, 2, 1024], F8), ("dbg_ex3", [P, 2, 1024], F8),
            ("dbg_acc00", [P, 1024], F32), ("dbg_oz0", [P, S], F32),
        ):
            dbg[nm] = nc.dram_tensor(nm, shp, dt, kind="ExternalOutput")[:]
    with tile.TileContext(nc) as tc:
        _build_body(tc, xT[:], wqT[:], wkT[:], wvT[:], wcT[:], identT[:],
                    yT[:], dbg)
    nc.finalize()
    return nc


_NC_CACHE = None


def _get_nc():
    global _NC_CACHE
    if _NC_CACHE is None:
        _NC_CACHE = build_bass()
    return _NC_CACHE


def kernel(x, Wq, Wk, Wv, Wc, bc, **kw):
    import ml_dtypes
    bf = ml_dtypes.bfloat16
    f8 = ml_dtypes.float8_e4m3
    x = np.asarray(x, np.float32)
    ident = np.eye(P, dtype=bf)
    in_maps = []
    for c in range(NCORES):
        b, g = divmod(c, NCORES // B)
        sl = slice(g * EG, (g + 1) * EG)
        in_maps.append({
            "xT": np.ascontiguousarray(np.asarray(x[b]).T.astype(f8)),
            "wqT": np.ascontiguousarray(np.asarray(Wq)[sl, :].T.astype(f8)),
            "wkT": np.ascontiguousarray(np.asarray(Wk)[sl, :].T.astype(f8)),
            "wvT": np.ascontiguousarray(np.asarray(Wv)[sl, :].T.astype(f8)),
            "wcT": np.ascontiguousarray(np.asarray(Wc)[:, sl].T),
            "identT": ident,
        })
    nc = _get_nc()
    res = run_bass_kernel_spmd(
        nc, in_maps, core_ids=list(range(NCORES)),
        trace=bool(int(os.environ.get("QK_TRACE", "0"))),
    )
    y = np.zeros((B, S, E), np.float32)
    for c in range(NCORES):
        b = c // (NCORES // B)
        y[b] += res.results[c]["yT"].T
    y += np.asarray(bc, np.float32)
    globals()["_LAST_RESULT"] = res
    return y


# revision 54
# speedup vs baseline: 1.0047x; 1.0047x over previous
"""Trainium2 Bass kernel: multi-head attention with quantum (cumprod-of-cos) transform.

Full-input contract: kernel(**inputs) takes the unsharded inputs and returns the
full [B, S, E] output. Internally shards over 8 NeuronCores: data-parallel over
batch (B=2) x tensor-parallel over head-groups (4 heads per core).

Per-core pipeline (b = batch, g = head-group of 4 heads, EG = 256 e-dims):
  x8/w8: x and Wq/Wk/Wv arrive pre-quantized to fp8e4 from the host.
  theta [s, f] per t-tile: fp8 DoubleRow matmuls pairing the 8 contraction
      tiles (2 k-tiles per instruction, 2 fp8 rows/cycle on HW). The q
      projection runs as per-contraction-pair passes over all 16 t-tiles so
      the PE starts as soon as the first two x tiles land.
  c = cos(theta) via ACT Sin(theta + pi/2); all Sin ops precede all Exp ops
      in ACT program order -> exactly 2 activation-table loads.
  z = cumprod(c) along d via DVE tensor_tensor_scan (mult/bypass, fp32 state),
      one 64-wide scan per head: q,k -> bf16; v -> fp8e4 (+ fp8 ones columns
      interleaved for the free softmax denominator).
  qzT/kzT: PE transpose (bf16 identity matmul) [s,e] -> [e,s] between the k
      and v projections; PSUM->SBUF copies split across ACT and DVE.
  scores^T [t, s] per head/t-tile = kzT-tile (stationary) x qzT (moving), bf16.
  ex8 = exp(scores/8 + ln2/2) in fp8e4: 10 tiles/iter on ACT (spline exp, fp8
      out) and 6 on DVE (one tensor_scalar: bits = round(s/ln2 + 60) written
      as uint8 == the e4m3 bit pattern; |err| ~ fp8 quantization). Odd tiles
      on DVE so consecutive tiles drain on different engines.
  out-matmul: fp8 DoubleRow pairing two t-tiles per instruction; stationary
      [ones | vz8] -> acc rows 0:64 = softmax denominator (reciprocal_approx_
      fast needs partition-0-based input), rows 64:128 = unnormalized out^T.
      oz = acc[64:128] * recip(acc[0:64]) on DVE.
  yT_partial [E, S] = WcT_slice (stationary, f32r) x oz -- host sums 4
      partials. Software-pipelined attention: iteration i emits scores+exp,
      then iteration i-1's out-matmuls; the final projection of s-half 0 is
      split across two pipeline slots to smooth the PSUM rotation.
"""

import os
import sys

import numpy as np

if "/opt/trn_rl_repo" not in sys.path:
    sys.path.insert(0, "/opt/trn_rl_repo")

import concourse.bass as bass  # noqa: F401
import concourse.tile as tile
from concourse import bacc
from concourse import mybir
from concourse.bass_utils import run_bass_kernel_spmd

AF = mybir.ActivationFunctionType
ALU = mybir.AluOpType
F32 = mybir.dt.float32
F32R = mybir.dt.float32r
BF16 = mybir.dt.bfloat16
F8 = mybir.dt.float8e4
U8 = mybir.dt.uint8

B, S, E, H, D = 2, 2048, 1024, 16, 64
NCORES = 8
HG = 4          # heads per core
EG = HG * D     # 256
P = 128
NT = S // P     # 16 t-tiles
KC = E // P     # 8 contraction tiles for the projections
HALF_PI = float(np.pi / 2)
INV_SQRT_D = 0.125  # 1/sqrt(64)
# ex = exp(s/8 + ln2/2): raw scores measured in [-6.8, 11.3] so max ex = 5.9
# and the DVE bit pattern round(s*1/ln2 + 60) stays in [50, 77] -- far from
# the uint8 wrap at 0 and the e4m3 NaN zone above 126. The uniform 2^(1/2)
# scale cancels against the matching denominator.
EXP_SHIFT = float(np.log(2.0) / 2.0)
EXP_A = float(1.0 / np.log(2.0))
EXP_B = 60.0
# exp split: odd t-tiles on DVE so each (even, odd) pair overlaps ACT||DVE
_DVE_T = frozenset((1, 3, 5, 7, 9, 11))
_DEBUG = bool(int(os.environ.get("QK_DEBUG", "0")))


def _projections_q(tc, x8, w8q, c_pool, z_pool, zqk, psQ, hp):
    """q projection as contraction passes: pass j touches all 16 t-tiles, so
    the first matmuls need only x8[:, 0:2] instead of the full x8 -- the PE
    starts ~10us earlier while the remaining x tiles stream in."""
    nc = tc.nc
    for half in range(2):
        thh = psQ.tile([P, KC, EG], F32, tag="thh", bufs=2, name=f"thh{half}")
        for j in range(KC // 2):
            for ts in range(KC):
                t = half * KC + ts
                nc.tensor.matmul(
                    thh[:, ts, :],
                    lhsT=x8[:, 2 * j:2 * j + 2, t * P:(t + 1) * P],
                    rhs=w8q[:, 2 * j:2 * j + 2, :],
                    start=(j == 0), stop=(j == KC // 2 - 1),
                    perf_mode=mybir.MatmulPerfMode.DoubleRow,
                )
        for u in range(KC // 2):
            c_t = c_pool.tile([P, 2 * EG], F32, tag="c", name=f"cq{half}{u}")
            nc.scalar.activation(c_t[:], thh[:, 2 * u:2 * u + 2, :], AF.Sin,
                                 bias=hp[:])
            for sl in range(2):
                t = half * KC + 2 * u + sl
                zt = z_pool.tile([P, EG], BF16, tag=f"zq{t}", name=f"zq{t}")
                zqk[("q", t)] = zt
                for h in range(HG):
                    src = c_t[:, sl * EG + h * D:sl * EG + (h + 1) * D]
                    nc.vector.tensor_tensor_scan(
                        out=zt[:, h * D:(h + 1) * D], data0=src, data1=src,
                        initial=1.0, op0=ALU.mult, op1=ALU.bypass,
                    )


def _projections(tc, x8, w8s, c_pool, z_pool, vz8, zqk, ps, hp, names):
    """theta -> cos -> cumprod. Emits ACT Sin ops (all Sins precede all
    Exps across calls)."""
    nc = tc.nc
    for name in names:
        w8 = w8s[name]
        for tp in range(NT // 2):
            th = ps.tile([P, 2 * EG], F32, tag="th", bufs=4, name=f"th{name}{tp}")
            for sl in range(2):
                t = 2 * tp + sl
                for j in range(KC // 2):
                    nc.tensor.matmul(
                        th[:, sl * EG:(sl + 1) * EG],
                        lhsT=x8[:, 2 * j:2 * j + 2, t * P:(t + 1) * P],
                        rhs=w8[:, 2 * j:2 * j + 2, :],
                        start=(j == 0), stop=(j == KC // 2 - 1),
                        perf_mode=mybir.MatmulPerfMode.DoubleRow,
                    )
            c_t = c_pool.tile([P, 2 * EG], F32, tag="c", name=f"c{name}{tp}")
            nc.scalar.activation(c_t[:], th[:], AF.Sin, bias=hp[:])
            for sl in range(2):
                t = 2 * tp + sl
                for h in range(HG):
                    off = sl * EG + h * D
                    src = c_t[:, off:off + D]
                    if name == "v":
                        out = vz8[tp][:, sl, 2 * h + 1, :]
                    else:
                        zt = zqk[(name, t)] = (
                            zqk.get((name, t))
                            or z_pool.tile([P, EG], BF16, tag=f"z{name}{t}",
                                           name=f"z{name}{t}")
                        )
                        out = zt[:, h * D:(h + 1) * D]
                    nc.vector.tensor_tensor_scan(
                        out=out, data0=src, data1=src,
                        initial=1.0, op0=ALU.mult, op1=ALU.bypass,
                    )


def _transposes(tc, zqk, ident, zT, ps):
    """[s, e] bf16 tiles -> [e, s] via PE transpose; PSUM->SBUF copies split
    ACT/DVE. Runs between k- and v-projections so v's matmuls keep the PE fed
    while the last scans drain."""
    nc = tc.nc
    for name in ("q", "k"):
        for t in range(NT):
            for m in range(2):
                pt = ps.tile([P, P], BF16, tag="tp", bufs=4,
                             name=f"pt{name}{m}{t}")
                nc.tensor.matmul(
                    pt[:], lhsT=zqk[(name, t)][:, m * P:(m + 1) * P],
                    rhs=ident[:], is_transpose=True, start=True, stop=True,
                )
                eng = nc.scalar if m == 0 else nc.vector
                if eng is nc.scalar:
                    nc.scalar.copy(out=zT[(name, m)][:, t * P:(t + 1) * P],
                                   in_=pt[:])
                else:
                    nc.vector.tensor_copy(
                        out=zT[(name, m)][:, t * P:(t + 1) * P], in_=pt[:])


def _attention_and_final(tc, zT, vz8, wc_t, oz_pool, yT, psB, esh, dbg):
    nc = tc.nc
    with (
        tc.tile_pool(name="exps", bufs=3) as exq,
        tc.tile_pool(name="norm", bufs=2) as nrm,
        tc.tile_pool(name="y", bufs=3) as yp,
    ):
        ozs = {}

        def emit_outdr(sb, h, exs):
            # out-matmuls + normalize for a (sb, h) whose exps are in flight
            ssl0 = sb * 1024
            m = h // 2
            dbase = (h % 2) * D
            acc = psB.tile([P, 1024], F32, tag="acc", bufs=1, name=f"acc{h}_{sb}")
            for tp in range(NT // 2):
                for ch in range(2):
                    nc.tensor.matmul(
                        acc[:, ch * 512:(ch + 1) * 512],
                        lhsT=vz8[tp][:, :, 2 * h:2 * h + 2, :].rearrange(
                            "p a b d -> p a (b d)"),
                        rhs=exs[tp][:, :, ch * 512:(ch + 1) * 512],
                        start=(tp == 0), stop=(tp == NT // 2 - 1),
                        perf_mode=mybir.MatmulPerfMode.DoubleRow,
                    )
            if _DEBUG and sb == 0 and h == 0:
                accs = nrm.tile([P, 1024], F32, tag="accs", name=f"accs{h}{sb}")
                nc.vector.tensor_copy(out=accs[:], in_=acc[:])
                nc.sync.dma_start(out=dbg["dbg_acc00"][:], in_=accs[:])
            rec = nrm.tile([D, 1024], F32, tag="rec", name=f"rec{h}{sb}")
            nc.vector.reciprocal_approx_fast(rec[:], acc[0:D, :])
            nc.vector.tensor_tensor(
                out=ozs[m][dbase:dbase + D, ssl0:ssl0 + 1024],
                in0=acc[D:2 * D, :], in1=rec[:], op=ALU.mult,
            )

        def emit_final(sb, mos=range(E // P)):
            for mo in mos:
                py = psB.tile([P, 1024], F32, tag="s", bufs=3, name=f"py{mo}{sb}")
                for ch in range(2):
                    ssl = slice(sb * 1024 + ch * 512, sb * 1024 + (ch + 1) * 512)
                    for kk in range(2):
                        nc.tensor.matmul(
                            py[:, ch * 512:(ch + 1) * 512],
                            lhsT=wc_t[:, kk, mo * P:(mo + 1) * P],
                            rhs=ozs[kk][:, ssl],
                            start=(kk == 0), stop=(kk == 1),
                        )
                yt = yp.tile([P, 1024], F32, tag="y", name=f"yt{mo}{sb}")
                if mo % 2 == 0:
                    nc.scalar.copy(out=yt[:], in_=py[:])
                else:
                    nc.vector.tensor_copy(out=yt[:], in_=py[:])
                nc.sync.dma_start(
                    out=yT[mo * P:(mo + 1) * P, sb * 1024:(sb + 1) * 1024],
                    in_=yt[:],
                )

        prev = None
        iters = [(sb, h) for sb in range(2) for h in range(HG)]
        for i, (sb, h) in enumerate(iters):
            ssl0 = sb * 1024
            m = h // 2
            dbase = (h % 2) * D
            qzT = zT[("q", m)]
            kzT = zT[("k", m)]
            if sb == 0 and (h % 2) == 0:
                ozm = oz_pool.tile([P, S], F32R, tag=f"oz{m}", name=f"oz{m}")
                ozs[m] = ozm
                _OZ_HANDLES[m] = ozm
            exs = []
            for tp in range(NT // 2):
                ex = exq.tile([P, 2, 1024], F8, tag="ex", bufs=18,
                              name=f"ex{h}_{tp}{sb}")
                exs.append(ex)
                if _DEBUG and sb == 0 and h == 0 and tp in (0, 3):
                    dbg[("ex", tp)] = ex
                for sl in range(2):
                    t = 2 * tp + sl
                    sc = psB.tile([P, 1024], F32, tag="s", bufs=3,
                                  name=f"sc{h}_{t}{sb}")
                    for ch in range(2):
                        nc.tensor.matmul(
                            sc[:, ch * 512:(ch + 1) * 512],
                            lhsT=kzT[dbase:dbase + D, t * P:(t + 1) * P],
                            rhs=qzT[dbase:dbase + D,
                                    ssl0 + ch * 512:ssl0 + (ch + 1) * 512],
                            start=True, stop=True,
                        )
                    if t in _DVE_T:
                        # DVE path: e4m3 bit pattern via one affine+round
                        nc.vector.tensor_scalar(
                            out=ex[:, sl, :].bitcast(U8), in0=sc[:],
                            scalar1=EXP_A, scalar2=EXP_B,
                            op0=ALU.mult, op1=ALU.add,
                        )
                    else:
                        nc.scalar.activation(
                            ex[:, sl, :], sc[:], AF.Exp,
                            scale=INV_SQRT_D, bias=esh[:],
                        )
                if _DEBUG and sb == 0 and h == 0 and tp in (0, 3):
                    nc.sync.dma_start(out=dbg[f"dbg_ex{tp}"][:],
                                      in_=dbg[("ex", tp)][:])
            if prev is not None:
                emit_outdr(*prev)
            if i == 5:
                emit_final(0, range(4))    # sb 0 done after iteration 4's outdr
            elif i == 6:
                emit_final(0, range(4, 8))
            prev = (sb, h, exs)
        emit_outdr(*prev)
        emit_final(1)


_OZ_HANDLES = {}


def _build_body(tc, xT, wqT, wkT, wvT, wcT, identT, yT, dbg):
    nc = tc.nc

    with (
        tc.tile_pool(name="const", bufs=1) as const,
        tc.tile_pool(name="wc", bufs=1) as wcp,
        tc.tile_pool(name="x8p", bufs=1) as x8p,
        tc.tile_pool(name="w8p", bufs=1) as w8p,
        tc.tile_pool(name="vz", bufs=1) as vzp,
        tc.tile_pool(name="zT", bufs=1) as zTp,
        tc.tile_pool(name="outz", bufs=1) as ozp,
    ):
        ident = const.tile([P, P], BF16)
        nc.sync.dma_start(out=ident[:], in_=identT[:])
        hp = const.tile([P, 1], F32)
        nc.vector.memset(hp[:], HALF_PI)
        esh = const.tile([P, 1], F32)
        nc.vector.memset(esh[:], EXP_SHIFT)
        wc_t = wcp.tile([P, 2, E], F32R)

        x8 = x8p.tile([P, KC, S], F8)
        w8s = {}
        vz8 = [vzp.tile([P, 2, 2 * HG, D], F8, tag=f"vz{tp}", name=f"vz{tp}")
               for tp in range(NT // 2)]
        zT = {(name, m): zTp.tile([P, S], BF16, tag=f"zT{name}{m}",
                                  name=f"zT{name}{m}")
              for name in ("q", "k") for m in range(2)}

        with (
            tc.tile_pool(name="ctile", bufs=3) as cp,
            tc.tile_pool(name="ztile", bufs=1) as zp,
        ):
            # x and q/k/v weights arrive pre-quantized to fp8 from the host:
            # q weights first (the first projection matmul needs them), then
            # the x tiles in contraction order
            w8_q = w8p.tile([P, KC, EG], F8, tag="w8q", name="w8_q")
            nc.sync.dma_start(
                out=w8_q[:], in_=wqT.rearrange("(k p) e -> p k e", p=P))
            w8s["q"] = w8_q
            for k in range(KC):
                nc.sync.dma_start(out=x8[:, k, :], in_=xT[k * P:(k + 1) * P, :])
            for name, wT in (("k", wkT), ("v", wvT)):
                w8 = w8p.tile([P, KC, EG], F8, tag=f"w8{name}", name=f"w8{name}")
                w8s[name] = w8
                nc.sync.dma_start(
                    out=w8[:], in_=wT.rearrange("(k p) e -> p k e", p=P))

            nc.sync.dma_start(out=wc_t[:],
                              in_=wcT.rearrange("(k p) e -> p k e", p=P))
            # fp8 ones columns for the softmax denominator; ones FIRST within
            # each head pair so the denominator lands on out partitions 0:64
            # (reciprocal_approx_fast mishandles partition-offset inputs)
            for tp in range(NT // 2):
                nc.gpsimd.memset(vz8[tp][:, :, 0:2 * HG:2, :], 1.0)

            zqk = {}
            with tc.tile_pool(name="psQ", bufs=1, space="PSUM") as psQ:
                _projections_q(tc, x8, w8s["q"], cp, zp, zqk, psQ, hp)
            with tc.tile_pool(name="psA", bufs=1, space="PSUM") as psA:
                _projections(tc, x8, w8s, cp, zp, vz8, zqk, psA, hp, ("k",))
                _transposes(tc, zqk, ident, zT, psA)
                _projections(tc, x8, w8s, cp, zp, vz8, zqk, psA, hp, ("v",))
            if _DEBUG:
                nc.sync.dma_start(out=dbg["dbg_x8"][:], in_=x8[:])
                nc.sync.dma_start(out=dbg["dbg_w8q"][:], in_=w8s["q"][:])
                nc.sync.dma_start(out=dbg["dbg_zq0"][:], in_=zqk[("q", 0)][:])
                nc.sync.dma_start(out=dbg["dbg_zTq0"][:], in_=zT[("q", 0)][:])
                nc.sync.dma_start(out=dbg["dbg_vz0"][:], in_=vz8[0][:])

        with tc.tile_pool(name="psB", bufs=1, space="PSUM") as psB:
            _attention_and_final(tc, zT, vz8, wc_t, ozp, yT, psB, esh, dbg)
            if _DEBUG:
                nc.sync.dma_start(out=dbg["dbg_oz0"][:],
                                  in_=_OZ_HANDLES[0][:].bitcast(F32))


def build_bass():
    nc = bacc.Bacc(None, target_bir_lowering=False)
    xT = nc.dram_tensor("xT", [E, S], F8, kind="ExternalInput")
    wqT = nc.dram_tensor("wqT", [E, EG], F8, kind="ExternalInput")
    wkT = nc.dram_tensor("wkT", [E, EG], F8, kind="ExternalInput")
    wvT = nc.dram_tensor("wvT", [E, EG], F8, kind="ExternalInput")
    wcT = nc.dram_tensor("wcT", [EG, E], F32R, kind="ExternalInput")
    identT = nc.dram_tensor("identT", [P, P], BF16, kind="ExternalInput")
    yT = nc.dram_tensor("yT", [E, S], F32, kind="ExternalOutput")
    dbg = {}
    if _DEBUG:
        for nm, shp, dt in (
            ("dbg_x8", [P, KC, S], F8), ("dbg_w8q", [P, KC, EG], F8),
            ("dbg_zq0", [P, EG], BF16), ("dbg_zTq0", [P, S], BF16),
            ("dbg_vz0", [P, 2, 2 * HG, D], F8),
            ("dbg_ex0", [P, 2, 1024], F8), ("dbg_ex3", [P, 2, 1024], F8),
            ("dbg_acc00", [P, 1024], F32), ("dbg_oz0", [P, S], F32),
        ):
            dbg[nm] = nc.dram_tensor(nm, shp, dt, kind="ExternalOutput")[:]
    with tile.TileContext(nc) as tc:
        _build_body(tc, xT[:], wqT[:], wkT[:], wvT[:], wcT[:], identT[:],
                    yT[:], dbg)
    nc.finalize()
    return nc


_NC_CACHE = None


def _get_nc():
    global _NC_CACHE
    if _NC_CACHE is None:
        _NC_CACHE = build_bass()
    return _NC_CACHE


def kernel(x, Wq, Wk, Wv, Wc, bc, **kw):
    import ml_dtypes
    bf = ml_dtypes.bfloat16
    f8 = ml_dtypes.float8_e4m3
    x = np.asarray(x, np.float32)
    ident = np.eye(P, dtype=bf)
    in_maps = []
    for c in range(NCORES):
        b, g = divmod(c, NCORES // B)
        sl = slice(g * EG, (g + 1) * EG)
        in_maps.append({
            "xT": np.ascontiguousarray(np.asarray(x[b]).T.astype(f8)),
            "wqT": np.ascontiguousarray(np.asarray(Wq)[sl, :].T.astype(f8)),
            "wkT": np.ascontiguousarray(np.asarray(Wk)[sl, :].T.astype(f8)),
            "wvT": np.ascontiguousarray(np.asarray(Wv)[sl, :].T.astype(f8)),
            "wcT": np.ascontiguousarray(np.asarray(Wc)[:, sl].T),
            "identT": ident,
        })
    nc = _get_nc()
    res = run_bass_kernel_spmd(
        nc, in_maps, core_ids=list(range(NCORES)),
        trace=bool(int(os.environ.get("QK_TRACE", "0"))),
    )
    y = np.zeros((B, S, E), np.float32)
    for c in range(NCORES):
        b = c // (NCORES // B)
        y[b] += res.results[c]["yT"].T
    y += np.asarray(bc, np.float32)
    globals()["_LAST_RESULT"] = res
    return y
